# revision 1
# baseline (speedup 1.0000x reference)
"""LIF (leaky integrate-and-fire) scan kernel for Trainium2, 8 NeuronCores.

Reference semantics (fp32, T=8 innermost axis):
    mem = 0
    for t in range(T):
        mem = mem * 0.5 + x[..., t]
        s[..., t] = (mem >= 1.0)
        mem = mem * (1.0 - s[..., t])

Sharding: data-parallel over the leading dim (64 -> 8 per core). On the host,
each core's shard is transposed to a t-major layout [128 partitions, T=8,
8192 neurons] so that every per-timestep slice the device touches is
contiguous (strided SBUF reads measured ~2x slower on DVE, and strided writes
block the 2x tensor_scalar mode).

Per chunk of neurons, all on the Vector engine (exact in fp32):
    m    = (m  mult 0.5) add x_t       # scalar_tensor_tensor, 1x
    x_t  = (m  is_ge 1.0)              # tensor_scalar spike, 2x, in place
    m    = (m  is_lt 1.0) mult m       # scalar_tensor_tensor reset, 1x
Each timestep's strip is loaded/stored with its own ~1 MiB DMA so loads,
compute, and stores pipeline at strip granularity.
"""

import numpy as np

import concourse.bass as bass
import concourse.tile as tile
from concourse import bacc, mybir
from concourse.bass_utils import run_bass_kernel_spmd

P = 128          # SBUF partitions
T = 8            # timesteps (innermost axis of the original input)
NPB = 8192       # neurons per partition per core: 8*128*32*32 / 128
FREE = NPB * T   # fp32 elements per partition per core
CH = 2048        # neurons per chunk (per partition)
NCH = NPB // CH

THRESH = 1.0
DECAY = 0.5
F32 = mybir.dt.float32
N_CORES = 8

Alu = mybir.AluOpType


def _build() -> bass.Bass:
    nc = bacc.Bacc("TRN2", target_bir_lowering=False, debug=False)
    # t-major per core: x[p, t*NPB + n]
    x = nc.dram_tensor("x", [P, FREE], F32, kind="ExternalInput").ap()
    y = nc.dram_tensor("y", [P, FREE], F32, kind="ExternalOutput").ap()

    with tile.TileContext(nc) as tc:
        with (
            tc.tile_pool(name="strips", bufs=8) as strips,
            tc.tile_pool(name="state", bufs=2) as state,
        ):
            for c in range(NCH):
                xs = []
                for t in range(T):
                    st = strips.tile([P, CH], F32, tag="strip", name=f"st{c}_{t}")
                    nc.gpsimd.dma_start(
                        st[:], x[:, t * NPB + c * CH : t * NPB + (c + 1) * CH]
                    )
                    xs.append(st)
                m = state.tile([P, CH], F32, tag="m", name=f"m{c}")
                for t in range(T):
                    st = xs[t]
                    if t == 0:
                        # mem0 = 0, so m = x_0 after decay+add.
                        nc.vector.scalar_tensor_tensor(
                            m[:], st[:], THRESH, st[:], Alu.is_lt, Alu.mult
                        )
                        nc.vector.tensor_scalar(
                            st[:], st[:], THRESH, None, Alu.is_ge, Alu.bypass
                        )
                    else:
                        nc.vector.scalar_tensor_tensor(
                            m[:], m[:], DECAY, st[:], Alu.mult, Alu.add
                        )
                        nc.vector.tensor_scalar(
                            st[:], m[:], THRESH, None, Alu.is_ge, Alu.bypass
                        )
                        if t < T - 1:
                            nc.vector.scalar_tensor_tensor(
                                m[:], m[:], THRESH, m[:], Alu.is_lt, Alu.mult
                            )
                    nc.gpsimd.dma_start(
                        y[:, t * NPB + c * CH : t * NPB + (c + 1) * CH], st[:]
                    )
    nc.compile()
    return nc


_NC_CACHE: bass.Bass | None = None


def _get_nc() -> bass.Bass:
    global _NC_CACHE
    if _NC_CACHE is None:
        _NC_CACHE = _build()
    return _NC_CACHE


def _run(X: np.ndarray, **spmd_kwargs):
    assert X.shape == (64, 128, 32, 32, 8), X.shape
    X = np.ascontiguousarray(X, dtype=np.float32)
    per_core = 64 // N_CORES
    # [core, p, n, t] -> t-major [core, p, t, n], contiguous per core
    Xt = np.ascontiguousarray(
        X.reshape(N_CORES, P, NPB, T).transpose(0, 1, 3, 2)
    )
    in_maps = [{"x": Xt[i].reshape(P, FREE)} for i in range(N_CORES)]
    res = run_bass_kernel_spmd(
        _get_nc(), in_maps, core_ids=list(range(N_CORES)), **spmd_kwargs
    )
    out = np.empty_like(X)
    for i, r in enumerate(res.results):
        # t-major [p, t, n] -> [p, n, t] -> original shard shape
        s = r["y"].reshape(P, T, NPB).transpose(0, 2, 1)
        out[i * per_core : (i + 1) * per_core] = s.reshape(
            per_core, 128, 32, 32, 8
        )
    return out, res


def kernel(X: np.ndarray) -> np.ndarray:
    out, _ = _run(X)
    return out



# revision 4
# speedup vs baseline: 1.1334x; 1.1334x over previous
"""LIF (leaky integrate-and-fire) scan kernel for Trainium2, 8 NeuronCores.

Reference semantics (fp32, T=8 innermost axis):
    mem = 0
    for t in range(T):
        mem = mem * 0.5 + x[..., t]
        s[..., t] = (mem >= 1.0)
        mem = mem * (1.0 - s[..., t])

Strategy (vs the fp32 baseline at ~218 us):
  * int16 fixed-point (scale 2^12): x is quantized on the host; membrane m
    stays int16 on device. DVE computes fp32 internally, so the only error
    sources are the one-time x quantization and the per-step int16 rounding
    of m (quantum 2^-12). Empirically 2005 spike flips vs the fp32 reference
    = rel err 0.0143 < 2e-2 (all fp32->i16 rounding modes pass: rne 0.0143,
    trunc 0.0117, floor 0.0105, ceil 0.0166). 16-bit dtypes unlock the DVE
    2x_1P mode for both scalar_tensor_tensor ops and halve input DMA.
  * Packed u8 output: the 8 spikes of a neuron become one byte. The spike
    planes sigma_t = Sign(m - 4095.5) in {-1,+1} (ACT engine) are packed by
    the Tensor engine: 8 accumulating matmuls with diagonal weights 2^(t-1)
    give sum_t 2^(t-1) sigma_t = packed_byte - 127.5 in PSUM; the +127.5 is
    applied in the ACT psum->sbuf u8 copy. Output DMA drops 32x to 1 MiB.
  * Per-core HBM traffic: 16 MiB in + 1 MiB out (~50 us at 358 GB/s) vs
    64 MiB (187 us) for the fp32 baseline.

Per-core layout: data-parallel over the leading dim (64 -> 8 per core),
t-major strips [128 partitions, T=8, 8192 neurons] so every per-timestep
slice is contiguous (keeps DVE 2x mode and dense DMA descriptors).

Engine split per timestep (1M neurons/core, CH=2048 chunks):
    DVE  A: m   = (m_prev mult 0.5) add x_t      stt, i16, 2x
    ACT  B: sg  = Sign(m - 4095.5) -> bf16       never 0: m integer, bias .5
    PE   C: psum[b] += W_t^T @ sg  (W_t = 2^(t-1) I)
    DVE  D: m'  = (m is_lt 4096) mult m          stt, i16, 2x  (skip t=7)
"""

import numpy as np

import concourse.bass as bass
import concourse.tile as tile
from concourse import bacc, mybir
from concourse.bass_utils import run_bass_kernel_spmd

P = 128          # SBUF partitions
T = 8            # timesteps (innermost axis of the original input)
NPB = 8192       # neurons per partition per core: 8*128*32*32 / 128
CH = 2048        # neurons per chunk (per partition)
NCH = NPB // CH
PSB = 512        # psum bank free size (fp32)
NB = CH // PSB   # psum banks per chunk

SCALE = 4096.0   # fixed-point scale 2^12
THR = 4096.0     # threshold 1.0 in scaled units
N_CORES = 8

F32 = mybir.dt.float32
I16 = mybir.dt.int16
U8 = mybir.dt.uint8
BF16 = mybir.dt.bfloat16

Alu = mybir.AluOpType
Act = mybir.ActivationFunctionType


def _build() -> bass.Bass:
    nc = bacc.Bacc("TRN2", target_bir_lowering=False, debug=False)
    x = nc.dram_tensor("x", [P, T, NPB], I16, kind="ExternalInput").ap()
    w = nc.dram_tensor("w", [P, T * P], BF16, kind="ExternalInput").ap()
    y = nc.dram_tensor("y", [P, NPB], U8, kind="ExternalOutput").ap()

    with tile.TileContext(nc) as tc:
        with (
            tc.tile_pool(name="xin", bufs=2) as xin,
            tc.tile_pool(name="mem", bufs=8) as mem,
            tc.tile_pool(name="spk", bufs=6) as spk,
            tc.tile_pool(name="acc", bufs=2) as accp,
            tc.tile_pool(name="wts", bufs=1) as wts,
            tc.tile_pool(name="ps", bufs=2, space="PSUM") as psp,
        ):
            wt = wts.tile([P, T * P], BF16, tag="w", name="wt")
            nc.sync.dma_start(wt[:], w[:, :])
            bias_s = wts.tile([P, 1], F32, tag="bs", name="bias_s")
            nc.vector.memset(bias_s[:], -(THR - 0.5))
            bias_p = wts.tile([P, 1], F32, tag="bp", name="bias_p")
            nc.vector.memset(bias_p[:], 127.5)
            for c in range(NCH):
                xt = xin.tile([P, T, CH], I16, tag="x", name=f"x{c}")
                nc.sync.dma_start(xt[:], x[:, :, c * CH : (c + 1) * CH])
                ps = [
                    psp.tile([P, PSB], F32, tag=f"ps{b}", name=f"ps{c}_{b}")
                    for b in range(NB)
                ]
                cur = xt[:, 0, :]  # m_0 = x_0 (mem starts at 0): alias, no copy
                prev = None
                for t in range(T):
                    if t > 0:
                        nxt = mem.tile([P, CH], I16, tag="m", name=f"m{c}_{t}")
                        nc.vector.scalar_tensor_tensor(
                            nxt[:], prev[:], 0.5, xt[:, t, :], Alu.mult, Alu.add
                        )
                        cur = nxt[:]
                    sg = spk.tile([P, CH], BF16, tag="s", name=f"s{c}_{t}")
                    nc.scalar.activation(
                        sg[:], cur, Act.Sign, bias=bias_s[:], scale=1.0
                    )
                    for b in range(NB):
                        nc.tensor.matmul(
                            ps[b][:],
                            wt[:, t * P : (t + 1) * P],
                            sg[:, b * PSB : (b + 1) * PSB],
                            start=(t == 0),
                            stop=(t == T - 1),
                        )
                    if t < T - 1:
                        rst = mem.tile([P, CH], I16, tag="m", name=f"r{c}_{t}")
                        nc.vector.scalar_tensor_tensor(
                            rst[:], cur, THR, cur, Alu.is_lt, Alu.mult
                        )
                        prev = rst
                acc = accp.tile([P, CH], U8, tag="a", name=f"a{c}")
                for b in range(NB):
                    nc.scalar.activation(
                        acc[:, b * PSB : (b + 1) * PSB],
                        ps[b][:],
                        Act.Identity,
                        bias=bias_p[:],
                        scale=1.0,
                    )
                nc.sync.dma_start(y[:, c * CH : (c + 1) * CH], acc[:])
    nc.compile()
    return nc


_NC_CACHE: bass.Bass | None = None


def _get_nc() -> bass.Bass:
    global _NC_CACHE
    if _NC_CACHE is None:
        _NC_CACHE = _build()
    return _NC_CACHE


def _weights() -> np.ndarray:
    # W_t = 2^(t-1) * I, laid out as [P, T*P] (lhsT slices [128, 128] per t).
    wf = np.zeros((P, T * P), dtype=np.float32)
    for t in range(T):
        wf[:, t * P : (t + 1) * P][np.arange(P), np.arange(P)] = 2.0 ** (t - 1)
    return wf.astype(mybir.dt.np(BF16))


def _run(X: np.ndarray, **spmd_kwargs):
    assert X.shape == (64, 128, 32, 32, 8), X.shape
    X = np.asarray(X, dtype=np.float32)
    per_core = 64 // N_CORES
    q = np.clip(np.rint(X * SCALE), -32768.0, 32767.0).astype(np.int16)
    # [core, p, n, t] -> t-major [core, p, t, n], contiguous per core
    qt = np.ascontiguousarray(
        q.reshape(N_CORES, P, NPB, T).transpose(0, 1, 3, 2)
    )
    wnp = _weights()
    in_maps = [{"x": qt[i], "w": wnp} for i in range(N_CORES)]
    res = run_bass_kernel_spmd(
        _get_nc(), in_maps, core_ids=list(range(N_CORES)), **spmd_kwargs
    )
    out = np.empty_like(X)
    for i, r in enumerate(res.results):
        packed = r["y"].reshape(P, NPB, 1).astype(np.uint8)
        bits = np.unpackbits(packed, axis=2, bitorder="little")  # [P, NPB, 8]
        out[i * per_core : (i + 1) * per_core] = bits.astype(np.float32).reshape(
            per_core, 128, 32, 32, 8
        )
    return out, res


def kernel(X: np.ndarray) -> np.ndarray:
    out, _ = _run(X)
    return out


# revision 5
# speedup vs baseline: 1.7278x; 1.5244x over previous
"""LIF (leaky integrate-and-fire) scan kernel for Trainium2, 8 NeuronCores.

Reference semantics (fp32, T=8 innermost axis):
    mem = 0
    for t in range(T):
        mem = mem * 0.5 + x[..., t]
        s[..., t] = (mem >= 1.0)
        mem = mem * (1.0 - s[..., t])

Strategy (fp32 baseline: ~218 us; v1 int16+stt+ACT-sign: ~193 us):
  * int16 fixed-point (scale 2^12): x quantized on the host, membrane M
    int16 on device. DVE ALUs compute fp32 internally, so the only error
    sources are x quantization and one int16 rounding of M per step
    (quantum 2^-12): 2005 spike flips vs the fp32 reference = rel err
    0.0143 < 2e-2 (robust to HW rounding mode: rne 0.0143, trunc 0.0117,
    floor 0.0105, ceil 0.0166).
  * HW-measured op selection (FD=2048/partition): scalar_tensor_tensor is
    ALWAYS 1x (2284 ns) regardless of dtype; tensor_tensor on 16-bit is 2x
    (1225 ns); tensor_scalar 16-bit in/out is 4x (694 ns). So the update is
    restructured to avoid stt entirely, and to fuse decay+reset in one
    tensor_tensor via a {0, 0.5}-valued mask:
        A: M_t  = tt_add(M'_{t-1}, y_t)            int16, 2x   (y = x*4096)
        B: r2_t = ts(M_t is_lt 4096) mult 0.5      fp16 {0,.5}, 4x
        D: M'_t = tt_mult(M_t, r2_t)               int16, 2x = reset AND decay
    ~3144 ns DVE per step per 2048-chunk; spike mask r2 doubles as the
    packing input.
  * Packed u8 output (32x less output traffic): PE accumulates 8 matmuls
    with diagonal fp16 weights -2^(t+1) over the r2 planes into PSUM:
    psum = -sum_t 2^t r_t = packed_byte - 255; ACT adds 255 during the
    PSUM -> SBUF u8 copy. byte bit t = spike at step t.
  * Per-core HBM traffic: 16 MiB in + 1 MiB out (~50 us at 358 GB/s).

Per-core layout: data-parallel over the leading dim (64 -> 8 per core),
t-major strips [128 partitions, T=8, 8192 neurons]; all ops touch
contiguous [128, 2048] strips (keeps DVE 2x/4x modes + dense DMA).
"""

import numpy as np

import concourse.bass as bass
import concourse.tile as tile
from concourse import bacc, mybir
from concourse.bass_utils import run_bass_kernel_spmd

P = 128          # SBUF partitions
T = 8            # timesteps (innermost axis of the original input)
NPB = 8192       # neurons per partition per core: 8*128*32*32 / 128
CH = 2048        # neurons per chunk (per partition)
NCH = NPB // CH
PSB = 512        # psum bank free size (fp32)
NB = CH // PSB   # psum banks per chunk

SCALE = 4096.0   # fixed-point scale 2^12
THR = 4096.0     # threshold 1.0 in scaled units
N_CORES = 8

F32 = mybir.dt.float32
I16 = mybir.dt.int16
U8 = mybir.dt.uint8
F16 = mybir.dt.float16

Alu = mybir.AluOpType
Act = mybir.ActivationFunctionType


def _build() -> bass.Bass:
    nc = bacc.Bacc("TRN2", target_bir_lowering=False, debug=False)
    x = nc.dram_tensor("x", [P, T, NPB], I16, kind="ExternalInput").ap()
    w = nc.dram_tensor("w", [P, T * P], F16, kind="ExternalInput").ap()
    y = nc.dram_tensor("y", [P, NPB], U8, kind="ExternalOutput").ap()

    with tile.TileContext(nc) as tc:
        with (
            tc.tile_pool(name="xin", bufs=2) as xin,
            tc.tile_pool(name="mem", bufs=6) as mem,
            tc.tile_pool(name="msk", bufs=6) as msk,
            tc.tile_pool(name="acc", bufs=2) as accp,
            tc.tile_pool(name="wts", bufs=1) as wts,
            tc.tile_pool(name="ps", bufs=2, space="PSUM") as psp,
        ):
            wt = wts.tile([P, T * P], F16, tag="w", name="wt")
            nc.sync.dma_start(wt[:], w[:, :])
            bias_p = wts.tile([P, 1], F32, tag="bp", name="bias_p")
            nc.vector.memset(bias_p[:], 255.0)
            for c in range(NCH):
                xt = xin.tile([P, T, CH], I16, tag="x", name=f"x{c}")
                nc.sync.dma_start(xt[:], x[:, :, c * CH : (c + 1) * CH])
                ps = [
                    psp.tile([P, PSB], F32, tag=f"ps{b}", name=f"ps{c}_{b}")
                    for b in range(NB)
                ]
                cur = xt[:, 0, :]  # M_0 = y_0 (mem starts at 0): alias, no copy
                for t in range(T):
                    if t > 0:
                        nxt = mem.tile([P, CH], I16, tag="m", name=f"m{c}_{t}")
                        nc.vector.tensor_tensor(
                            nxt[:], prev[:], xt[:, t, :], Alu.add
                        )
                        cur = nxt[:]
                    r2 = msk.tile([P, CH], F16, tag="r", name=f"r{c}_{t}")
                    nc.vector.tensor_scalar(
                        r2[:], cur, THR, 0.5, Alu.is_lt, Alu.mult
                    )
                    for b in range(NB):
                        nc.tensor.matmul(
                            ps[b][:],
                            wt[:, t * P : (t + 1) * P],
                            r2[:, b * PSB : (b + 1) * PSB],
                            start=(t == 0),
                            stop=(t == T - 1),
                        )
                    if t < T - 1:
                        rst = mem.tile([P, CH], I16, tag="m", name=f"d{c}_{t}")
                        nc.vector.tensor_tensor(rst[:], cur, r2[:], Alu.mult)
                        prev = rst
                acc = accp.tile([P, CH], U8, tag="a", name=f"a{c}")
                for b in range(NB):
                    nc.scalar.activation(
                        acc[:, b * PSB : (b + 1) * PSB],
                        ps[b][:],
                        Act.Identity,
                        bias=bias_p[:],
                        scale=1.0,
                    )
                nc.sync.dma_start(y[:, c * CH : (c + 1) * CH], acc[:])
    nc.compile()
    return nc


_NC_CACHE: bass.Bass | None = None


def _get_nc() -> bass.Bass:
    global _NC_CACHE
    if _NC_CACHE is None:
        _NC_CACHE = _build()
    return _NC_CACHE


def _weights() -> np.ndarray:
    # W_t = -2^(t+1) * I, laid out as [P, T*P] (lhsT slices [128, 128] per t).
    # psum = sum_t W_t^T r2_t = -sum_t 2^t r_t = packed_byte - 255.
    wf = np.zeros((P, T * P), dtype=np.float32)
    for t in range(T):
        wf[:, t * P : (t + 1) * P][np.arange(P), np.arange(P)] = -(2.0 ** (t + 1))
    return wf.astype(np.float16)


def _run(X: np.ndarray, **spmd_kwargs):
    assert X.shape == (64, 128, 32, 32, 8), X.shape
    X = np.asarray(X, dtype=np.float32)
    per_core = 64 // N_CORES
    q = np.clip(np.rint(X * SCALE), -32768.0, 32767.0).astype(np.int16)
    # [core, p, n, t] -> t-major [core, p, t, n], contiguous per core
    qt = np.ascontiguousarray(
        q.reshape(N_CORES, P, NPB, T).transpose(0, 1, 3, 2)
    )
    wnp = _weights()
    in_maps = [{"x": qt[i], "w": wnp} for i in range(N_CORES)]
    res = run_bass_kernel_spmd(
        _get_nc(), in_maps, core_ids=list(range(N_CORES)), **spmd_kwargs
    )
    out = np.empty_like(X)
    for i, r in enumerate(res.results):
        packed = r["y"].reshape(P, NPB, 1).astype(np.uint8)
        bits = np.unpackbits(packed, axis=2, bitorder="little")  # [P, NPB, 8]
        out[i * per_core : (i + 1) * per_core] = bits.astype(np.float32).reshape(
            per_core, 128, 32, 32, 8
        )
    return out, res


def kernel(X: np.ndarray) -> np.ndarray:
    out, _ = _run(X)
    return out


# revision 6
# speedup vs baseline: 2.0027x; 1.1591x over previous
"""LIF (leaky integrate-and-fire) scan kernel for Trainium2, 8 NeuronCores.

Reference semantics (fp32, T=8 innermost axis):
    mem = 0
    for t in range(T):
        mem = mem * 0.5 + x[..., t]
        s[..., t] = (mem >= 1.0)
        mem = mem * (1.0 - s[..., t])

Strategy (fp32 baseline: ~218 us; v1 int16+stt+ACT-sign: ~193 us; v3
stt-free: ~126 us):
  * int16 fixed-point (scale 2^12): x quantized on the host, membrane M
    int16 on device. DVE ALUs compute fp32 internally, so the only error
    sources are x quantization and one int16 rounding of M per step
    (quantum 2^-12): ~2e3 spike flips vs the fp32 reference = rel err
    ~0.014 < 2e-2 (robust to HW rounding mode: rne 0.0143, trunc 0.0117,
    floor 0.0105, ceil 0.0166).
  * HW-measured op selection (per 2048 elems/partition):
    scalar_tensor_tensor is ALWAYS 1x (2284 ns) regardless of dtype;
    tensor_tensor on 16-bit is 2x (1225 ns); tensor_scalar 16-bit in/out
    is 4x (694 ns); mask dtype must be fp16 (int16 out is pathological,
    bf16 mixed-TT drops to ~1.3x). The update avoids stt entirely and
    fuses decay+reset into one tensor_tensor via a {0, 0.5} mask:
        A: M_t  = tt_add(M'_{t-1}, y_t)            int16, 2x   (y = x*4096)
        B: r2_t = ts(M_t is_lt 4096) mult 0.5      fp16 {0,.5}, 4x
        D: M'_t = tt_mult(M_t, r2_t)               int16, 2x = reset AND decay
  * Packed u8 output (32x less output traffic): PE accumulates 8 matmuls
    with diagonal fp16 weights -2^(t+1) over the r2 planes into PSUM:
    psum = -sum_t 2^t r_t = packed_byte - 255; ACT adds 255 during the
    PSUM -> SBUF u8 copy. byte bit t = spike at step t.
  * Per-core HBM traffic: 16 MiB in + 1 MiB out (~50 us at 358 GB/s).
  * CH=4096 chunks (2 per core) amortize per-instruction overhead; input
    arrives as 8 per-timestep 1 MiB DMAs per chunk so compute starts
    ~3 us in (v3 waited 15 us on one 4 MiB chunk DMA); output leaves as
    two 256 KiB DMAs per chunk to shorten the tail.

Per-core layout: data-parallel over the leading dim (64 -> 8 per core),
t-major strips [128 partitions, T=8, 8192 neurons]; all compute touches
contiguous [128, 4096] strips (keeps DVE 2x/4x modes + dense DMA).
"""

import numpy as np

import concourse.bass as bass
import concourse.tile as tile
from concourse import bacc, mybir
from concourse.bass_utils import run_bass_kernel_spmd

P = 128          # SBUF partitions
T = 8            # timesteps (innermost axis of the original input)
NPB = 8192       # neurons per partition per core: 8*128*32*32 / 128
CH = 4096        # neurons per chunk (per partition)
NCH = NPB // CH
PSB = 512        # psum bank free size (fp32)
NB = CH // PSB   # psum banks per chunk

SCALE = 4096.0   # fixed-point scale 2^12
THR = 4096.0     # threshold 1.0 in scaled units
N_CORES = 8

F32 = mybir.dt.float32
I16 = mybir.dt.int16
U8 = mybir.dt.uint8
F16 = mybir.dt.float16

Alu = mybir.AluOpType
Act = mybir.ActivationFunctionType


def _build() -> bass.Bass:
    nc = bacc.Bacc("TRN2", target_bir_lowering=False, debug=False)
    x = nc.dram_tensor("x", [P, T, NPB], I16, kind="ExternalInput").ap()
    w = nc.dram_tensor("w", [P, T * P], F16, kind="ExternalInput").ap()
    y = nc.dram_tensor("y", [P, NPB], U8, kind="ExternalOutput").ap()

    with tile.TileContext(nc) as tc:
        with (
            tc.tile_pool(name="xin", bufs=10) as xin,
            tc.tile_pool(name="mem", bufs=4) as mem,
            tc.tile_pool(name="msk", bufs=5) as msk,
            tc.tile_pool(name="acc", bufs=2) as accp,
            tc.tile_pool(name="wts", bufs=1) as wts,
            tc.tile_pool(name="ps", bufs=1, space="PSUM") as psp,
        ):
            wt = wts.tile([P, T * P], F16, tag="w", name="wt")
            nc.sync.dma_start(wt[:], w[:, :])
            bias_p = wts.tile([P, 1], F32, tag="bp", name="bias_p")
            nc.vector.memset(bias_p[:], 255.0)
            for c in range(NCH):
                lo = c * CH
                xs = []
                for t in range(T):
                    st = xin.tile([P, CH], I16, tag="x", name=f"x{c}_{t}")
                    nc.sync.dma_start(st[:], x[:, t, lo : lo + CH])
                    xs.append(st)
                ps = [
                    psp.tile([P, PSB], F32, tag=f"ps{b}", name=f"ps{c}_{b}")
                    for b in range(NB)
                ]
                acc = accp.tile([P, CH], U8, tag="a", name=f"a{c}")
                cur = xs[0][:]  # M_0 = y_0 (mem starts at 0): alias, no copy
                for t in range(T):
                    if t > 0:
                        nxt = mem.tile([P, CH], I16, tag="m", name=f"m{c}_{t}")
                        nc.vector.tensor_tensor(nxt[:], prev[:], xs[t][:], Alu.add)
                        cur = nxt[:]
                    r2 = msk.tile([P, CH], F16, tag="r", name=f"r{c}_{t}")
                    nc.vector.tensor_scalar(
                        r2[:], cur, THR, 0.5, Alu.is_lt, Alu.mult
                    )
                    for b in range(NB):
                        nc.tensor.matmul(
                            ps[b][:],
                            wt[:, t * P : (t + 1) * P],
                            r2[:, b * PSB : (b + 1) * PSB],
                            start=(t == 0),
                            stop=(t == T - 1),
                        )
                    if t < T - 1:
                        rst = mem.tile([P, CH], I16, tag="m", name=f"d{c}_{t}")
                        nc.vector.tensor_tensor(rst[:], cur, r2[:], Alu.mult)
                        prev = rst
                for b in range(NB):
                    nc.scalar.activation(
                        acc[:, b * PSB : (b + 1) * PSB],
                        ps[b][:],
                        Act.Identity,
                        bias=bias_p[:],
                        scale=1.0,
                    )
                    if b % 4 == 3:
                        nc.sync.dma_start(
                            y[:, lo + (b - 3) * PSB : lo + (b + 1) * PSB],
                            acc[:, (b - 3) * PSB : (b + 1) * PSB],
                        )
    nc.compile()
    return nc


_NC_CACHE: bass.Bass | None = None


def _get_nc() -> bass.Bass:
    global _NC_CACHE
    if _NC_CACHE is None:
        _NC_CACHE = _build()
    return _NC_CACHE


def _weights() -> np.ndarray:
    # W_t = -2^(t+1) * I, laid out as [P, T*P] (lhsT slices [128, 128] per t).
    # psum = sum_t W_t^T r2_t = -sum_t 2^t r_t = packed_byte - 255.
    wf = np.zeros((P, T * P), dtype=np.float32)
    for t in range(T):
        wf[:, t * P : (t + 1) * P][np.arange(P), np.arange(P)] = -(2.0 ** (t + 1))
    return wf.astype(np.float16)


def _run(X: np.ndarray, **spmd_kwargs):
    assert X.shape == (64, 128, 32, 32, 8), X.shape
    X = np.asarray(X, dtype=np.float32)
    per_core = 64 // N_CORES
    q = np.clip(np.rint(X * SCALE), -32768.0, 32767.0).astype(np.int16)
    # [core, p, n, t] -> t-major [core, p, t, n], contiguous per core
    qt = np.ascontiguousarray(
        q.reshape(N_CORES, P, NPB, T).transpose(0, 1, 3, 2)
    )
    wnp = _weights()
    in_maps = [{"x": qt[i], "w": wnp} for i in range(N_CORES)]
    res = run_bass_kernel_spmd(
        _get_nc(), in_maps, core_ids=list(range(N_CORES)), **spmd_kwargs
    )
    out = np.empty_like(X)
    for i, r in enumerate(res.results):
        packed = r["y"].reshape(P, NPB, 1).astype(np.uint8)
        bits = np.unpackbits(packed, axis=2, bitorder="little")  # [P, NPB, 8]
        out[i * per_core : (i + 1) * per_core] = bits.astype(np.float32).reshape(
            per_core, 128, 32, 32, 8
        )
    return out, res


def kernel(X: np.ndarray) -> np.ndarray:
    out, _ = _run(X)
    return out
